# revision 55
# baseline (speedup 1.0000x reference)
"""Trainium2 Bass kernel for the DISL loss (nn_DISL_Loss).

Strategy (data-parallel over batch rows, 8 cores, fp8/bf16 compute):
  Host: cast v/oa/of/vaf to fp8e4m3 (loss tolerance is 2e-2; measured
    end-to-end error of the low-precision pipeline is ~2e-4 because the
    greedy matching is degenerate and the loss aggregates over 16K rows).
  Pass 1 (device, per core): G_A = OA^T V and G_F = OF^T V partials via
    fp8 DoubleRow matmuls (4x bf16 rate in the cost model), plus the
    triplet row-sums Tm = W^T vaf (W prescaled x512 so fp8 never
    underflows; the scale cancels under normalization). Five PSUM
    "quarters" rotate through one pool so copies/output DMA overlap the
    next quarter; loads are spread over the three DMA queues (they
    transfer concurrently).
  Host: all-reduce the G partials, sim = G / ||V_col|| (only V's column
    norms matter: row scaling of sim is argmax-invariant), greedy unique
    assignment (tiny, sequential), inverse-permutation index vectors.
    Row norms rnV/rnA/rnF are cheap O(n) host reductions.
    n1 = <OA_row, (V @ QA)_row>, n2 = <OF_row, (V @ QF)_row>,
    n3 = <OA_row, (OF @ Qg)_row> -- exactly equivalent to the reference's
    padded-permutation cosine numerators (verified numerically).
  Pass 2 (device, transposed [feature, row] layout): VgA^T/VgF^T come
    from dma_gather (SWDGE row-gathers straight from HBM by runtime
    int16 indices, prepared on two queues + triggered, consumers wait on
    the completion semaphores); OFg^T via PE DoubleRow one-hot matmuls
    (overlapping the gathers). Products are 2x DVE multiplies on SBUF
    bf16; the per-row reduction contracts partitions on PE ones-matmuls.
  Host: cos/CE/BCE/triplet final combine (small tensors only).
"""

import numpy as np
import ml_dtypes

B, T, M, OM = 64, 256, 1024, 512
N_CORES = 8
SPC = B // N_CORES          # samples per core
RPC = SPC * T               # rows per core
P = 128
KT = RPC // P               # row-tiles per core
WSCALE = 512.0

F8 = ml_dtypes.float8_e4m3
BF = ml_dtypes.bfloat16

_prog_cache = {}


# ---------------------------------------------------------------- pass 1
def _build_pass1(rows):
    from concourse import bacc, mybir
    from concourse.tile import TileContext

    f32 = mybir.dt.float32
    fp8 = mybir.dt.float8e4
    DR = mybir.MatmulPerfMode.DoubleRow
    kt = rows // P
    npair = kt // 2

    nc = bacc.Bacc()
    v_d = nc.declare_dram_parameter("v8", [rows, M], fp8, isOutput=False)
    oa_d = nc.declare_dram_parameter("oa8", [rows, OM], fp8, isOutput=False)
    of_d = nc.declare_dram_parameter("of8", [rows, OM], fp8, isOutput=False)
    vaf_d = nc.declare_dram_parameter("vaf8", [rows, M], fp8, isOutput=False)
    # W is [rows, 4] logically; padded to 128 cols so the DoubleRow Tm
    # matmul has a full (128,128) ldweights tile (ISA check rejects col<64)
    w_d = nc.declare_dram_parameter("w8", [rows, P], fp8, isOutput=False)
    ga_d = nc.declare_dram_parameter("ga", [OM, M], fp8, isOutput=True)
    gf_d = nc.declare_dram_parameter("gf", [OM, M], fp8, isOutput=True)
    tm_d = nc.declare_dram_parameter("tm", [4, M], f32, isOutput=True)

    with TileContext(nc) as tc:
        with (
            tc.tile_pool(name="res", bufs=1) as rpool,
            tc.tile_pool(name="out", bufs=2) as opool,
        ):
            v_sb = rpool.tile([P, kt, M], fp8, tag="v")
            oa_sb = rpool.tile([P, kt, OM], fp8, tag="oa")
            of_sb = rpool.tile([P, kt, OM], fp8, tag="of")
            vaf_sb = rpool.tile([P, kt, M], fp8, tag="vaf")
            w_sb = rpool.tile([P, kt, P], fp8, tag="w")

            # loads spread over four DMA queues (they transfer concurrently)
            def ld1(dram, sb, i, chunks, eng):
                per = kt // chunks
                eng.dma_start(
                    sb[:, i * per:(i + 1) * per, :],
                    dram[i * per * P:(i + 1) * per * P, :].rearrange(
                        "(k p) m -> p k m", p=P))

            def ldr(dram, sb, k0, k1, eng):
                eng.dma_start(
                    sb[:, k0:k1, :],
                    dram[k0 * P:k1 * P, :].rearrange(
                        "(k p) m -> p k m", p=P))

            ldr(oa_d, oa_sb, 0, 2, nc.sync)
            ldr(v_d, v_sb, 0, 2, nc.gpsimd)
            ldr(v_d, v_sb, 2, 6, nc.scalar)
            ldr(oa_d, oa_sb, 2, 9, nc.sync)
            ldr(v_d, v_sb, 6, 11, nc.gpsimd)
            ldr(v_d, v_sb, 11, 16, nc.scalar)
            ldr(oa_d, oa_sb, 9, 16, nc.sync)
            ldr(of_d, of_sb, 0, 8, nc.gpsimd)
            ldr(of_d, of_sb, 8, 16, nc.sync)
            ldr(vaf_d, vaf_sb, 0, 8, nc.scalar)
            ldr(vaf_d, vaf_sb, 8, 16, nc.sync)
            nc.gpsimd.dma_start(
                w_sb[:], w_d.rearrange("(k p) c -> p k c", p=P))

            gab = rpool.tile([P, 4, M], fp8, tag="gab")
            gfb = rpool.tile([P, 4, M], fp8, tag="gfb")

            # five "quarters" rotating through one PSUM pool (bufs=2):
            # G_A mc01, G_A mc23, Tm, G_F mc01, G_F mc23.  Each quarter's
            # psum->sbuf copies and output DMA overlap the next quarter.
            with tc.tile_pool(name="psq", bufs=2, space="PSUM") as psq:
                def g_quarter(src_sb, mch, gsb, g_dram, tagc):
                    tiles = []
                    for nh in range(2):
                        gp = psq.tile([P, 2, 512], f32, tag=f"q{nh}",
                                      name=f"g{tagc}{mch}_{nh}")
                        tiles.append(gp)
                    for jp in range(npair):
                        for mi in range(2):
                            mc = mch * 2 + mi
                            for nh in range(2):
                                nc.tensor.matmul(
                                    tiles[nh][:, mi, :],
                                    lhsT=src_sb[:, 2 * jp:2 * jp + 2,
                                                mc * P:(mc + 1) * P],
                                    rhs=v_sb[:, 2 * jp:2 * jp + 2,
                                             nh * 512:(nh + 1) * 512],
                                    start=(jp == 0), stop=(jp == npair - 1),
                                    perf_mode=DR)
                    lo = mch * 2
                    nc.scalar.copy(gsb[:, lo:lo + 2, 0:512], tiles[0][:])
                    nc.vector.tensor_copy(gsb[:, lo:lo + 2, 512:M],
                                          tiles[1][:])
                    eng = nc.sync if mch == 0 else nc.gpsimd
                    eng.dma_start(
                        g_dram[mch * 256:(mch + 1) * 256, :].rearrange(
                            "(c p) m -> p c m", p=P),
                        gsb[:, lo:lo + 2, :])

                g_quarter(oa_sb, 0, gab, ga_d, "a")
                g_quarter(oa_sb, 1, gab, ga_d, "a")

                # Tm quarter: both nh halves in one [P, 2, 512] tile
                tmps = psq.tile([P, 2, 512], f32, tag="q0", name="tmq")
                for jp in range(npair):
                    for nh in range(2):
                        nc.tensor.matmul(
                            tmps[:, nh, :],
                            lhsT=w_sb[:, 2 * jp:2 * jp + 2, :],
                            rhs=vaf_sb[:, 2 * jp:2 * jp + 2,
                                       nh * 512:(nh + 1) * 512],
                            start=(jp == 0), stop=(jp == npair - 1),
                            perf_mode=DR)
                tmo = opool.tile([4, 2, 512], f32, tag="tmo")
                nc.scalar.copy(tmo[:], tmps[0:4, :, :])
                nc.sync.dma_start(
                    tm_d.rearrange("a (b m) -> a b m", b=2), tmo[:])

                g_quarter(of_sb, 0, gfb, gf_d, "f")
                g_quarter(of_sb, 1, gfb, gf_d, "f")
    nc.finalize()
    return nc


# ---------------------------------------------------------------- pass 2
def _build_pass2(rows):
    from concourse import bacc, mybir
    from concourse.tile import TileContext
    from concourse.library_config import mlp

    f32 = mybir.dt.float32
    bf16 = mybir.dt.bfloat16
    fp8 = mybir.dt.float8e4
    i16 = mybir.dt.int16
    DR = mybir.MatmulPerfMode.DoubleRow
    RB = rows // 512            # 512-row stat blocks

    nc = bacc.Bacc(num_swdge_queues=2)
    # gather source stays in HBM (never DMA'd whole)
    vtb_d = nc.declare_dram_parameter("vtb", [M, rows], bf16, isOutput=False)
    oatb_d = nc.declare_dram_parameter("oatb", [OM, rows], bf16,
                                       isOutput=False)
    oftb_d = nc.declare_dram_parameter("oftb", [OM, rows], bf16,
                                       isOutput=False)
    oft8_d = nc.declare_dram_parameter("oft8", [OM, rows], fp8,
                                       isOutput=False)
    qg_d = nc.declare_dram_parameter("qg8", [OM, OM], fp8, isOutput=False)
    ixa_d = nc.declare_dram_parameter("ixa", [P, OM // 16], i16,
                                      isOutput=False)
    ixf_d = nc.declare_dram_parameter("ixf", [P, OM // 16], i16,
                                      isOutput=False)
    nst_d = nc.declare_dram_parameter("nst", [3, RB, 512], f32,
                                      isOutput=True)

    with TileContext(nc) as tc:
        with (
            tc.tile_pool(name="res", bufs=1) as rpool,
            tc.tile_pool(name="scr", bufs=3) as spool,
        ):
            oat_sb = rpool.tile([P, 4, rows], bf16, tag="oat")
            oftb_sb = rpool.tile([P, 4, rows], bf16, tag="oftb")
            oft8_sb = rpool.tile([P, 4, rows], fp8, tag="oft8")
            qg_sb = rpool.tile([P, 4, OM], fp8, tag="qg")
            ixa = rpool.tile([P, OM // 16], i16, tag="ixa")
            ixf = rpool.tile([P, OM // 16], i16, tag="ixf")
            ga_sb = rpool.tile([P, 4, rows], bf16, tag="ga")
            gf_sb = rpool.tile([P, 4, rows], bf16, tag="gf")
            onesb = rpool.tile([P, 1], bf16, tag="onesb")
            nc.vector.memset(onesb[:], 1.0)

            nc.gpsimd.load_library(mlp)
            nc.gpsimd.dma_start(ixa[:], ixa_d[:, :])
            nc.gpsimd.dma_start(ixf[:], ixf_d[:, :])
            gsems = [nc.alloc_semaphore(f"gsem{q}") for q in range(2)]
            nc.gpsimd.dma_gather(ga_sb[:], vtb_d[:, :], ixa[:], OM, OM,
                                 rows, prepare_only=True, sem=gsems[0],
                                 queue_num=0)
            nc.gpsimd.dma_gather(gf_sb[:], vtb_d[:, :], ixf[:], OM, OM,
                                 rows, prepare_only=True, sem=gsems[1],
                                 queue_num=1)
            nc.gpsimd.trigger_dma(count=None, queue_num=0)
            nc.gpsimd.trigger_dma(count=None, queue_num=1)
            # n3 operands and the first oat chunk load first (its compute
            # overlaps the gathers); chunks split across both HWDGE queues
            nc.sync.dma_start(
                qg_sb[:], qg_d.rearrange("(c p) m -> p c m", p=P))
            nc.sync.dma_start(
                oat_sb[:, 0:2, :],
                oatb_d[0:2 * P, :].rearrange("(c p) r -> p c r", p=P))
            for i in range(2):
                nc.scalar.dma_start(
                    oft8_sb[:, i * 2:(i + 1) * 2, :],
                    oft8_d[i * 2 * P:(i + 1) * 2 * P, :].rearrange(
                        "(c p) r -> p c r", p=P))
            nc.scalar.dma_start(
                oat_sb[:, 2:4, :],
                oatb_d[2 * P:4 * P, :].rearrange("(c p) r -> p c r", p=P))
            nc.sync.dma_start(
                oftb_sb[:, 0:2, :],
                oftb_d[0:2 * P, :].rearrange("(c p) r -> p c r", p=P))
            nc.scalar.dma_start(
                oftb_sb[:, 2:4, :],
                oftb_d[2 * P:4 * P, :].rearrange("(c p) r -> p c r", p=P))

            # n3 via PE DoubleRow one-hot matmuls (runs during the gathers),
            # n1/n2 stream as their gathers land (explicit DVE waits on the
            # gather-completion semaphores; the preps only signal desc-gen)
            with (
                tc.tile_pool(name="pgo", bufs=1, space="PSUM") as pgo,
                tc.tile_pool(name="pst", bufs=1, space="PSUM") as pst,
                tc.tile_pool(name="pst2", bufs=2, space="PSUM") as pst2,
            ):
                for rb in range(RB):
                    r0 = rb * 512
                    stat3 = pst.tile([1, 512], f32, tag="s3",
                                     name=f"s3_{rb}")
                    for ccp in range(2):
                        og = pgo.tile([P, 2, 512], f32, tag="og",
                                      name=f"og{rb}_{ccp}")
                        for s in range(2):
                            cc = 2 * ccp + s
                            for i in range(2):
                                nc.tensor.matmul(
                                    og[:, s, :],
                                    lhsT=qg_sb[:, 2 * i:2 * i + 2,
                                               cc * P:(cc + 1) * P],
                                    rhs=oft8_sb[:, 2 * i:2 * i + 2,
                                                r0:r0 + 512],
                                    start=(i == 0), stop=(i == 1),
                                    perf_mode=DR)
                        pr3 = spool.tile([P, 2, 512], bf16, tag="pr3",
                                         name=f"pr3{rb}_{ccp}")
                        nc.vector.tensor_mul(
                            pr3[:], oat_sb[:, 2 * ccp:2 * ccp + 2,
                                           r0:r0 + 512], og[:])
                        for s in range(2):
                            nc.tensor.matmul(
                                stat3[:], lhsT=onesb[:], rhs=pr3[:, s, :],
                                start=(ccp == 0 and s == 0),
                                stop=(ccp == 1 and s == 1))
                    so3 = spool.tile([1, 512], f32, tag="so3",
                                     name=f"so3_{rb}")
                    nc.scalar.copy(so3[:], stat3[:])
                    nc.gpsimd.dma_start(nst_d[2, rb:rb + 1, :], so3[:])

                for slot, (mt, g_sb, nm, sem) in enumerate((
                    (oat_sb, ga_sb, "a", gsems[0]),
                    (oftb_sb, gf_sb, "f", gsems[1]),
                )):
                    nc.vector.wait_ge(sem, 16)
                    for rb in range(RB):
                        r0 = rb * 512
                        stat = pst2.tile([1, 512], f32, tag=f"s{nm}",
                                         name=f"s{nm}{rb}")
                        pr = spool.tile([P, 4, 512], bf16, tag=f"pr{nm}",
                                        name=f"pr{nm}{rb}")
                        nc.vector.tensor_mul(
                            pr[:], mt[:, :, r0:r0 + 512],
                            g_sb[:, :, r0:r0 + 512])
                        for cc in range(4):
                            nc.tensor.matmul(
                                stat[:], lhsT=onesb[:], rhs=pr[:, cc, :],
                                start=(cc == 0), stop=(cc == 3))
                        so = spool.tile([1, 512], f32, tag=f"so{nm}",
                                        name=f"so{nm}{rb}")
                        nc.scalar.copy(so[:], stat[:])
                        eng = nc.sync if slot == 0 else nc.gpsimd
                        eng.dma_start(nst_d[slot, rb:rb + 1, :], so[:])
    nc.finalize()
    return nc


# ---------------------------------------------------------------- host math
def _greedy_ext(sim):
    om, m = sim.shape
    used = np.zeros(m, dtype=bool)
    I = np.empty(om, dtype=np.int32)
    for r in range(om):
        row = np.where(used, -np.inf, sim[r])
        c = int(np.argmax(row))
        I[r] = c
        used[c] = True
    ext = np.empty(m, dtype=np.int32)
    ext[:om] = I
    ext[om:] = np.nonzero(~used)[0]
    return ext


def _triplet_weights(label, seq_len, vaf_avf):
    f32 = np.float32
    y = np.asarray(label).astype(np.int64)
    n_idx = np.nonzero(y == 0)[0]
    a_idx = np.nonzero(y == 1)[0]
    W = np.zeros((B, T, 4), f32)
    ar = np.arange(T)
    Nn, Na = len(n_idx), len(a_idx)
    if Nn and Na:
        for b in n_idx:
            L = int(seq_len[b])
            W[b, :, 0] = (ar < L).astype(f32) * WSCALE / (f32(L) * Nn)
        for b in a_idx:
            L = int(seq_len[b])
            k = L // 16 + 1
            sig = np.asarray(vaf_avf[b], np.float64)
            valid = ar < L
            o_s = np.argsort(np.where(valid, sig, np.inf), kind="stable")
            o_l = np.argsort(np.where(valid, -sig, np.inf), kind="stable")
            W[b, o_s[:k], 1] = WSCALE / (f32(k) * Na)
            W[b, o_l[:k], 2] = WSCALE / (f32(k) * Na)
    return W, Nn, Na


_runner_cache = {}


def _make_runner(nc):
    """Cached variant of bass2jax.run_bass_via_pjrt's multi-core path: jit
    once per program, reuse the compiled executable across kernel() calls."""
    import jax
    import numpy as _np
    from jax.experimental.shard_map import shard_map
    from jax.sharding import Mesh, PartitionSpec
    from concourse import bass2jax, mybir

    bass2jax.install_neuronx_cc_hook()
    assert nc.dbg_addr is None or not nc.dbg_callbacks
    partition_name = (nc.partition_id_tensor.name
                      if nc.partition_id_tensor else None)
    in_names, out_names, out_avals, zero_shapes = [], [], [], []
    for alloc in nc.m.functions[0].allocations:
        if not isinstance(alloc, mybir.MemoryLocationSet):
            continue
        name = alloc.memorylocations[0].name
        if alloc.kind == "ExternalInput":
            if name != partition_name:
                in_names.append(name)
        elif alloc.kind == "ExternalOutput":
            shape = tuple(alloc.tensor_shape)
            dtype = mybir.dt.np(alloc.dtype)
            out_names.append(name)
            out_avals.append(jax.core.ShapedArray(shape, dtype))
            zero_shapes.append((shape, dtype))
    n_params = len(in_names)
    n_outs = len(out_names)
    all_in = list(in_names) + list(out_names)
    if partition_name is not None:
        all_in.append(partition_name)
    donate = tuple(range(n_params, n_params + n_outs))

    def _body(*args):
        operands = list(args)
        if partition_name is not None:
            operands.append(bass2jax.partition_id_tensor())
        return tuple(bass2jax._bass_exec_p.bind(
            *operands,
            out_avals=tuple(out_avals),
            in_names=tuple(all_in),
            out_names=tuple(out_names),
            lowering_input_output_aliases=(),
            sim_require_finite=True,
            sim_require_nnan=True,
            nc=nc,
        ))

    devices = jax.devices()[:N_CORES]
    mesh = Mesh(_np.asarray(devices), ("core",))
    in_specs = (PartitionSpec("core"),) * (n_params + n_outs)
    out_specs = (PartitionSpec("core"),) * n_outs
    sharded = jax.jit(
        shard_map(_body, mesh=mesh, in_specs=in_specs, out_specs=out_specs,
                  check_rep=False),
        donate_argnums=donate, keep_unused=True)

    def run(in_maps):
        concat_in = [
            np.concatenate([np.asarray(m[name]) for m in in_maps], axis=0)
            for name in in_names
        ]
        concat_zeros = [
            np.zeros((N_CORES * s[0], *s[1:]), d) for (s, d) in zero_shapes
        ]
        out_arrs = sharded(*concat_in, *concat_zeros)
        return [
            {name: np.asarray(out_arrs[i]).reshape(
                N_CORES, *out_avals[i].shape)[c]
             for i, name in enumerate(out_names)}
            for c in range(N_CORES)
        ]

    return run


def _run_spmd(nc, in_maps):
    key = id(nc)
    if key not in _runner_cache:
        _runner_cache[key] = _make_runner(nc)
    return _runner_cache[key](in_maps)


def kernel(v_satt, va_satt, vf_satt, vaf_satt, v_avf, va_avf, vf_avf, vaf_avf,
           va_out, vf_out, vaf_out, lamda1, lamda2, lamda3, lamda4,
           label, seq_len):
    f32 = np.float32
    v8 = np.asarray(v_satt, f32).reshape(B * T, M).astype(F8)
    oa8 = np.asarray(va_satt, f32).reshape(B * T, OM).astype(F8)
    of8 = np.asarray(vf_satt, f32).reshape(B * T, OM).astype(F8)
    vaf8 = np.asarray(vaf_satt, f32).reshape(B * T, M).astype(F8)

    W, Nn, Na = _triplet_weights(label, seq_len, vaf_avf)
    w8 = np.zeros((B * T, P), F8)
    w8[:, 0:4] = W.reshape(B * T, 4).astype(F8)

    if "p1" not in _prog_cache:
        _prog_cache["p1"] = _build_pass1(RPC)
    if "p2" not in _prog_cache:
        _prog_cache["p2"] = _build_pass2(RPC)

    def sl(x, c):
        return x[c * RPC:(c + 1) * RPC]

    in1 = [
        dict(v8=sl(v8, c), oa8=sl(oa8, c), of8=sl(of8, c),
             vaf8=sl(vaf8, c), w8=sl(w8, c))
        for c in range(N_CORES)
    ]
    res1 = _run_spmd(_prog_cache["p1"], in1)

    G_A = np.zeros((OM, M), np.float64)
    G_F = np.zeros((OM, M), np.float64)
    Tm = np.zeros((4, M), np.float64)
    for r in res1:
        G_A += r["ga"].astype(np.float64)
        G_F += r["gf"].astype(np.float64)
        Tm += r["tm"]

    # norms: cheap O(n) scalar summaries, computed host-side from the same
    # fp8-rounded values the device consumes
    v8f = v8.astype(f32)
    oa8f = oa8.astype(f32)
    of8f = of8.astype(f32)
    sqV = np.square(v8f)
    nV = np.maximum(np.sqrt(sqV.sum(0)), 1e-12)
    rnV = np.sqrt(sqV.sum(1, dtype=np.float64))
    rnA = np.sqrt(np.square(oa8f).sum(1, dtype=np.float64))
    rnF = np.sqrt(np.square(of8f).sum(1, dtype=np.float64))
    extA = _greedy_ext((G_A / nV[None, :]).astype(f32))
    extF = _greedy_ext((G_F / nV[None, :]).astype(f32))

    # gather indices: VgA[:, c] = V[:, invA[c]];  OFg[:, c] = OF[:, g[c]]
    invA = np.empty(M, np.int64)
    invA[extA] = np.arange(M)
    invF = np.empty(M, np.int64)
    invF[extF] = np.arange(M)
    g = extF[invA[:OM]]

    vtb = np.ascontiguousarray(
        v8.astype(BF).reshape(N_CORES, RPC, M).transpose(0, 2, 1))
    oatb = np.ascontiguousarray(
        oa8.astype(BF).reshape(N_CORES, RPC, OM).transpose(0, 2, 1))
    oftb = np.ascontiguousarray(
        of8.astype(BF).reshape(N_CORES, RPC, OM).transpose(0, 2, 1))
    oft8 = np.ascontiguousarray(
        of8.reshape(N_CORES, RPC, OM).transpose(0, 2, 1))
    Qg = np.zeros((OM, OM), F8)
    selg = g < OM
    Qg[g[selg], np.arange(OM)[selg]] = 1.0

    def idx_tile(vals):
        ix16 = np.zeros((16, OM // 16), np.int16)
        for j in range(OM):
            ix16[j % 16, j // 16] = vals[j]
        return np.tile(ix16, (8, 1))  # replicated across the 8 Q7 cores

    ixa = idx_tile(invA[:OM])
    ixf = idx_tile(invF[:OM])

    in2 = [
        dict(vtb=vtb[c], oatb=oatb[c], oftb=oftb[c], oft8=oft8[c],
             qg8=Qg, ixa=ixa, ixf=ixf)
        for c in range(N_CORES)
    ]
    res2 = _run_spmd(_prog_cache["p2"], in2)
    nst = np.concatenate(
        [r["nst"].reshape(3, RPC) for r in res2], axis=1)  # [3, B*T]

    n1 = nst[0].astype(np.float64)
    n2 = nst[1].astype(np.float64)
    n3 = nst[2].astype(np.float64)

    def cos_term(num, rx, ry):
        den = np.maximum(rx * ry, 1e-8)
        return (1.0 - num / den).reshape(B, T).mean(1).sum()

    d_sum = (cos_term(n1, rnV, rnA) + cos_term(n2, rnV, rnF)
             + cos_term(n3, rnA, rnF)) / B

    ar = np.arange(T)
    seqm = (ar[None, :] < np.asarray(seq_len)[:, None]).astype(np.float64)
    Vs = np.asarray(v_avf, np.float64) * seqm
    As = np.asarray(va_avf, np.float64) * seqm
    Fs = np.asarray(vf_avf, np.float64) * seqm

    def ce(q, p):
        e = 1e-6
        q = np.clip(q, e, 1 - e)
        p = np.clip(p, e, 1 - e)
        return -(p * np.log(q) + (1 - p) * np.log(1 - q)).mean()

    ma_loss = d_sum + ce(Vs, As) + ce(Vs, Fs) + ce(As, Fs)

    yf = np.asarray(label).astype(np.float64)

    def bce(p, yy):
        p = np.asarray(p, np.float64)
        return -(yy * np.log(p) + (1 - yy) * np.log(1 - p)).mean()

    a_loss = bce(va_out, yf)
    f_loss = bce(vf_out, yf)
    raf_loss = bce(vaf_out, yf)

    if Nn == 0 or Na == 0:
        trip = 0.0
    else:
        anchor, pos, neg = Tm[0] / WSCALE, Tm[1] / WSCALE, Tm[2] / WSCALE
        nrm = lambda x: x / np.linalg.norm(x)
        a_, p_, g_ = nrm(anchor), nrm(pos), nrm(neg)
        d = lambda x, z: np.linalg.norm(x - z + 1e-6)
        trip = max(d(a_, p_) - d(a_, g_) + 5.0, 0.0)

    lam = [float(lamda1), float(lamda2), float(lamda3), float(lamda4)]
    total = (lam[0] * ma_loss + lam[1] * (a_loss + f_loss)
             + lam[2] * raf_loss + lam[3] * trip)
    return np.array([total, ma_loss, a_loss + f_loss, raf_loss, trip], f32)
